# revision 18
# baseline (speedup 1.0000x reference)
"""Trainium2 Bass kernel for nn_AttentionBlock (masked GroupNorm + jagged full attention).

Contract: kernel(**inputs) takes FULL unsharded inputs (as in reference.setup_inputs())
and returns the FULL [8, 1024, 512] fp32 output. Internally shards data-parallel over
the batch: sample b -> NeuronCore b (8 cores).

v3: fp8e4 (e4m3) DoubleRow matmuls for qkv/v/av/proj (2x contraction per instr, 0.5
cycles/row); bf16 pair-packed scores (2 concurrent PE row groups); exp on ScalarE at
1-ktile granularity with ping-pong PSUM buffers (sA/sB) so the Activation engine (the
roofline: ~1 col/cycle @1.2GHz over H*L*L elements) never stalls; cross-(pair,qc)
software pipelining (the av tail + softmax normalize of pair p run under the scores/exp
of pair p+1); batched DMAs issued from multiple engine queues to cut the launch lead-in.

Per-core dataflow (sample s):
  xmT bf16 [128,CT,L] -> GroupNorm stats (bn_stats + selector matmuls) -> xn8 e4m3
    -> qkT bf16 (DoubleRow fp8, bias on DVE)  -> v8 e4m3 (masked, + denominator row)
    -> per pair: scoresT[k,q] bf16 -> exp e4m3 (ScalarE) -> av^T DoubleRow fp8
    -> reciprocal (DVE) -> partition_broadcast (Pool) -> normalize -> attn8 e4m3
    -> proj DoubleRow fp8 + residual (x*mask + all biases, host-folded), DMA out.

Padded tokens: x host-masked to zero => xn there ~= -mean*rstd*gamma (tiny); v rows
zeroed; denominator mask-row zero. Padded q columns give ~uniform attention over valid
k (|out| ~1e-2, well under the gate); residual rows are zero.
"""

import numpy as np
import ml_dtypes
from contextlib import ExitStack

B, L, C, G, H = 8, 1024, 512, 32, 8
DH = C // H          # 64
CPG = C // G         # 16
EPS = 1e-5
NT = L // 128        # 8 token tiles
CT = C // 128        # 4 channel tiles
CT2 = C // 256       # 2 DoubleRow channel groups
QC = L // 512        # 2 query chunks

BF16 = ml_dtypes.bfloat16
E4M3 = ml_dtypes.float8_e4m3

_CACHE = {}


def _build():
    import concourse.bass as bass
    import concourse.tile as tile
    from concourse import bacc, mybir

    f32 = mybir.dt.float32
    bf16 = mybir.dt.bfloat16
    e4 = mybir.dt.float8e4
    Alu = mybir.AluOpType
    Act = mybir.ActivationFunctionType
    DR = mybir.MatmulPerfMode.DoubleRow

    nc = bacc.Bacc("TRN2", target_bir_lowering=False)

    # ---- per-core DRAM inputs (host-prepped, partition-major for single-descriptor DMA) ----
    xmT_d = nc.dram_tensor("xmT", [128, CT, L], bf16, kind="ExternalInput")
    xm_d = nc.dram_tensor("xmr", [128, NT, C], f32, kind="ExternalInput")  # x*mf + folded biases
    wqk_d = nc.dram_tensor("wqk", [128, CT2, 8, 2, 128], e4, kind="ExternalInput")
    wvp_d = nc.dram_tensor("wvp", [128, 2, CT2, 2, C], e4, kind="ExternalInput")  # [v|p] rhs tiles
    # cs128: gamma[0:4] | bqk[4:12] | vmask[12:20] | sel[20:148]
    cs128_d = nc.dram_tensor("cs128", [128, 20 + CT * G], f32, kind="ExternalInput")
    cs32_d = nc.dram_tensor("cs32", [G, C + 1], f32, kind="ExternalInput")  # selT | icnt
    out_d = nc.dram_tensor("out", [L, C], f32, kind="ExternalOutput")

    with tile.TileContext(nc) as tc, ExitStack() as ctx:
        pc = ctx.enter_context(tc.tile_pool(name="consts", bufs=1))
        pb = ctx.enter_context(tc.tile_pool(name="big", bufs=1))
        ps = ctx.enter_context(tc.tile_pool(name="psum", bufs=1, space="PSUM"))

        # ---- batched loads, spread across engine queues ----
        cs128 = pc.tile([128, 20 + CT * G], f32, tag="cs128", name="cs128")
        cs32 = pc.tile([G, C + 1], f32, tag="cs32", name="cs32")
        nc.scalar.dma_start(cs128[:], cs128_d[:, :])
        nc.scalar.dma_start(cs32[:], cs32_d[:, :])
        xmt_sb = pb.tile([128, CT, L], bf16, tag="xmT", name="xmT")
        nc.sync.dma_start(xmt_sb[:, 0:1, :], xmT_d[:, 0:1, :])
        nc.sync.dma_start(xmt_sb[:, 1:2, :], xmT_d[:, 1:2, :])
        nc.scalar.dma_start(xmt_sb[:, 2:3, :], xmT_d[:, 2:3, :])
        nc.scalar.dma_start(xmt_sb[:, 3:4, :], xmT_d[:, 3:4, :])
        wqk_sb = pc.tile([128, CT2, 8, 2, 128], e4, tag="wqk", name="wqk")
        nc.gpsimd.dma_start(wqk_sb[:], wqk_d[:, :, :, :, :])
        wvp_sb = pc.tile([128, 2, CT2, 2, C], e4, tag="wvp", name="wvp")
        nc.gpsimd.dma_start(wvp_sb[:], wvp_d[:, :, :, :, :])
        xm_sb = pb.tile([128, NT, C], f32, tag="xm", name="xm")
        nc.gpsimd.dma_start(xm_sb[:, 0:4, :], xm_d[:, 0:4, :])
        nc.gpsimd.dma_start(xm_sb[:, 4:8, :], xm_d[:, 4:8, :])

        gam = cs128[:, 0:CT]
        bqk = cs128[:, CT:CT + 8]
        vmask = cs128[:, 12:12 + NT]
        sel = [cs128[:, 20 + G * ct:20 + G * (ct + 1)] for ct in range(CT)]
        selT = cs32[:, 0:C]
        icnt = cs32[:, C:C + 1]

        # ---- Phase 1: GroupNorm (stats over valid tokens; zeros from host masking) ----
        smm = [pb.tile([128, 2], f32, tag=f"smm{t}", name=f"smm{t}") for t in range(CT)]
        ps_g = ps.tile([G, 2], f32, tag="pA", name="psg")
        for t in range(CT):
            bns = pb.tile([128, 2, 6], f32, tag="bns", name="bns")
            nc.vector.bn_stats(bns[:, 0, :], xmt_sb[:, t, 0:512])
            nc.vector.bn_stats(bns[:, 1, :], xmt_sb[:, t, 512:1024])
            mv = pb.tile([128, 2], f32, tag="mv", name="mv")
            nc.vector.bn_aggr(mv[:], bns[:])
            sq = pb.tile([128, 1], f32, tag="sq", name="sq")
            nc.vector.tensor_mul(sq[:], mv[:, 0:1], mv[:, 0:1])
            # smm = [sum(x), sum(x^2)] recovered from mean/var over all 1024 (incl. zeros)
            nc.vector.tensor_scalar(smm[t][:, 0:1], mv[:, 0:1], float(L), None, Alu.mult)
            nc.vector.tensor_scalar(smm[t][:, 1:2], mv[:, 1:2], sq[:, 0:1], float(L), Alu.add, Alu.mult)
        for t in range(CT):
            nc.tensor.matmul(ps_g[:], sel[t], smm[t][:], start=(t == 0), stop=(t == CT - 1))
        grp = pb.tile([G, 2], f32, tag="grp", name="grp")      # [mean_g, rstd_g]
        ex2 = pb.tile([G, 1], f32, tag="ex2", name="ex2")
        nc.vector.tensor_scalar(grp[:, 0:1], ps_g[:, 0:1], icnt, None, Alu.mult)
        nc.vector.tensor_scalar(ex2[:], ps_g[:, 1:2], icnt, None, Alu.mult)
        mm2 = pb.tile([G, 1], f32, tag="mm2", name="mm2")
        nc.vector.tensor_mul(mm2[:], grp[:, 0:1], grp[:, 0:1])
        var = pb.tile([G, 1], f32, tag="var", name="var")
        nc.vector.tensor_tensor(var[:], ex2[:], mm2[:], Alu.subtract)
        sd = pb.tile([G, 1], f32, tag="sd", name="sd")
        eps_sb = pb.tile([G, 1], f32, tag="eps", name="eps")
        nc.vector.memset(eps_sb[:], EPS)
        nc.scalar.activation(sd[:], var[:], Act.Sqrt, bias=eps_sb[:], scale=1.0)
        # preload the Exp activation table while GroupNorm finishes on DVE
        # (reads sd so the scheduler keeps it AFTER the Sqrt's table load)
        dummy = pb.tile([G, 1], f32, tag="dummy", name="dummy")
        nc.scalar.activation(dummy[:], sd[:], Act.Exp, bias=0.0, scale=1.0)
        nc.vector.reciprocal(grp[:, 1:2], sd[:])

        # xn8[c2][p, i, tok] = xn(channel 256*c2 + 128*i + p, tok) in e4m3
        xn8_sb = [pb.tile([128, 2, L], e4, tag=f"xn8_{c2}", name=f"xn8_{c2}") for c2 in range(CT2)]
        rg_sb = pb.tile([128, CT], f32, tag="rg", name="rg")
        chst = [pb.tile([128, 2], f32, tag=f"chst{t}", name=f"chst{t}") for t in range(CT)]
        for t in range(CT):
            ps_b = ps.tile([128, 2], f32, tag="pB", name="psb")
            nc.tensor.matmul(ps_b[:], selT[:, 128 * t:128 * (t + 1)], grp[:], start=True, stop=True)
            nc.vector.tensor_copy(chst[t][:], ps_b[:])
            nc.vector.tensor_mul(rg_sb[:, t:t + 1], chst[t][:, 1:2], gam[:, t:t + 1])
            nc.vector.tensor_scalar(xn8_sb[t // 2][:, t % 2, :], xmt_sb[:, t, :],
                                    chst[t][:, 0:1], rg_sb[:, t:t + 1],
                                    Alu.subtract, Alu.mult)

        qkT_sb = [pb.tile([128, L], bf16, tag=f"qkT{ot}", name=f"qkT{ot}") for ot in range(8)]
        # v8[p, g, u, h, d]: v of token 128*(2g+u)+p, head h, dh d; d==64 is the mask
        # row; stride 72 keeps the DoubleRow ldweights step 16B-aligned
        v8_sb = pb.tile([128, NT // 2, 2, H, 72], e4, tag="v8", name="v8")
        attn8_sb = [pb.tile([128, 2, L], e4, tag=f"attn8_{c2}", name=f"attn8_{c2}") for c2 in range(CT2)]
        bcast_sb = [pb.tile([64, 512], f32, tag=f"bcast{j}", name=f"bcast{j}", bufs=2) for j in range(2)]

        def emit_qk1(ot, qc):
            qs = slice(512 * qc, 512 * (qc + 1))
            pq = ps.tile([128, 512], f32, tag=("pA" if qc == 0 else "pB"), name="pq")
            for c2 in range(CT2):
                nc.tensor.matmul(pq[:], wqk_sb[:, c2, ot, :, :], xn8_sb[c2][:, :, qs],
                                 start=(c2 == 0), stop=(c2 == CT2 - 1), perf_mode=DR)
            nc.vector.tensor_scalar(qkT_sb[ot][:, qs], pq[:],
                                    bqk[:, ot:ot + 1], None, Alu.add)

        def emit_v1(kt):
            pv = ps.tile([128, 512], f32, tag=("pA" if kt % 2 == 0 else "pB"), name="pv")
            for c2 in range(CT2):
                nc.tensor.matmul(pv[:], xn8_sb[c2][:, :, 128 * kt:128 * (kt + 1)],
                                 wvp_sb[:, 0, c2, :, :],
                                 start=(c2 == 0), stop=(c2 == CT2 - 1), perf_mode=DR)
            g, u = kt // 2, kt % 2
            nc.vector.tensor_scalar(v8_sb[:, g, u, :, 0:DH],
                                    pv[:].rearrange("p (h d) -> p h d", h=H),
                                    vmask[:, kt:kt + 1], None, Alu.mult)
            nc.vector.tensor_copy(v8_sb[:, g, u, :, DH],
                                  vmask[:, kt:kt + 1].to_broadcast((128, H)))

        def emit_proj(qt, ptag, otag):
            po = ps.tile([128, 512], f32, tag=ptag, name="po")
            for c2 in range(CT2):
                nc.tensor.matmul(po[:], attn8_sb[c2][:, :, 128 * qt:128 * (qt + 1)],
                                 wvp_sb[:, 1, c2, :, :],
                                 start=(c2 == 0), stop=(c2 == CT2 - 1), perf_mode=DR)
            o_sb = pb.tile([128, C], f32, tag=otag, name=otag)
            nc.vector.tensor_add(o_sb[:], po[:], xm_sb[:, qt, :])
            nc.sync.dma_start(out_d[128 * qt:128 * (qt + 1), :], o_sb[:])

        def emit_attn(p, qc, fin_prev, pre=(), fillers={}):
            kT = qkT_sb[4 + p]
            qT = qkT_sb[p]
            qs = slice(512 * qc, 512 * (qc + 1))
            avs = [ps.tile([DH + 1, 512], f32, tag=("avA" if j == 0 else "avB"), name=f"av{j}")
                   for j in range(2)]
            # expT2[p, j, kt, q]: exp'd transposed scores for the two heads of this pair
            expT2 = pb.tile([128, 2, NT, 512], e4, tag="expT2", name="expT2", bufs=2)

            def sc_exp(kt):
                ks = slice(128 * kt, 128 * (kt + 1))
                s = ps.tile([128, 2, 512], f32, tag=("sA" if kt % 2 == 0 else "sB"), name="s")
                nc.tensor.matmul(s[:, 0, :], kT[0:64, ks], qT[0:64, qs], start=True, stop=True)
                nc.tensor.matmul(s[:, 1, :], kT[64:128, ks], qT[64:128, qs], start=True, stop=True)
                nc.scalar.activation(expT2[:, :, kt, :], s[:], Act.Exp, bias=0.0, scale=0.125)

            def av_group(g):
                for j in range(2):
                    h = 2 * p + j
                    nc.tensor.matmul(avs[j][:], v8_sb[:, g, :, h, 0:DH + 1],
                                     expT2[:, j, 2 * g:2 * g + 2, :],
                                     start=(g == 0), stop=(g == NT // 2 - 1), perf_mode=DR)

            # cross-stage software pipeline: finish work of the previous (pair, qc)
            # runs under this pair's first scores/exp; filler units (qk/v/proj) are
            # threaded one-per-ktile to keep the PE fed without delaying scores
            sc_exp(0)
            if len(pre) > 0:
                pre[0]()
            sc_exp(1)
            if len(pre) > 1:
                pre[1]()
            if fin_prev is not None:
                fin_prev()
            for kt in range(2, NT):
                sc_exp(kt)
                if kt in fillers:
                    fillers[kt]()
                if kt % 2 == 1:
                    av_group((kt - 3) // 2)   # g = 0,1,2 at kt = 3,5,7

            def finish():
                av_group(NT // 2 - 1)
                for j in range(2):
                    # custom DVE/Pool ops can't read PSUM: copy the denominator row out
                    den = pb.tile([1, 512], f32, tag=f"den{j}", name=f"den{j}", bufs=2)
                    nc.vector.tensor_copy(den[:], avs[j][DH:DH + 1, :])
                    rec = pb.tile([1, 512], f32, tag=f"rec{j}", name=f"rec{j}", bufs=2)
                    nc.vector.reciprocal_approx_fast(rec[:], den[:])
                    nc.gpsimd.partition_broadcast(bcast_sb[j][:], rec[:])
                    nc.vector.tensor_tensor(attn8_sb[p // 2][64 * j:64 * (j + 1), p % 2, qs],
                                            avs[j][0:DH, :], bcast_sb[j][:], Alu.mult)
            return finish

        def qkf(ot, qc):
            return lambda: emit_qk1(ot, qc)

        def vf(kt):
            return lambda: emit_v1(kt)

        def pjf(qt):
            return lambda: emit_proj(qt, "pA" if qt % 2 == 0 else "pB", f"o{qt % 2}")

        # ---- emission schedule ----
        for ot in (0, 4):
            for qc in range(QC):
                emit_qk1(ot, qc)
        fin = emit_attn(0, 0, None, fillers={k + 2: vf(k) for k in range(6)})
        fin = emit_attn(0, 1, fin, pre=[vf(6), vf(7)],
                        fillers={2: qkf(1, 0), 3: qkf(1, 1), 4: qkf(5, 0), 5: qkf(5, 1)})
        fin = emit_attn(1, 0, fin,
                        fillers={2: qkf(2, 0), 3: qkf(2, 1), 4: qkf(6, 0), 5: qkf(6, 1)})
        fin = emit_attn(1, 1, fin,
                        fillers={2: qkf(3, 0), 3: qkf(3, 1), 4: qkf(7, 0), 5: qkf(7, 1)})
        fin = emit_attn(2, 0, fin)
        fin = emit_attn(2, 1, fin)
        fin = emit_attn(3, 0, fin)
        fin = emit_attn(3, 1, fin,
                        fillers={4: pjf(0), 5: pjf(1), 6: pjf(2), 7: pjf(3)})
        fin()
        for qt, ptag, otag in [(4, "pA", "o0"), (5, "pB", "o1"),
                               (6, "sA", "o2"), (7, "sB", "o3")]:
            emit_proj(qt, ptag, otag)

    nc.compile()
    return nc


def _get_nc():
    if "nc" not in _CACHE:
        _CACHE["nc"] = _build()
    return _CACHE["nc"]


def _prep_weights(gamma, beta, Wqkv, bqkv, Wproj, bproj):
    """Host-side constant prep shared across cores."""
    W = np.asarray(Wqkv, np.float32)
    bq = np.asarray(bqkv, np.float32) + np.asarray(beta, np.float32) @ W   # fold beta
    Wp = np.asarray(Wproj, np.float32)
    bv = bq[2 * C:3 * C]
    # residual-side constant: bproj + bv @ Wproj (added to masked rows on host)
    resid_bias = np.asarray(bproj, np.float32) + bv @ Wp

    # DoubleRow lhsT/rhs tiles: contraction channel (c2, i, p) = 256*c2 + 128*i + p
    wqk = np.zeros((128, CT2, 8, 2, 128), E4M3)
    wvp = np.zeros((128, 2, CT2, 2, C), E4M3)
    for c2 in range(CT2):
        for i in range(2):
            rows = slice(256 * c2 + 128 * i, 256 * c2 + 128 * (i + 1))
            for ot in range(8):
                wqk[:, c2, ot, i, :] = W[rows, 128 * ot:128 * (ot + 1)].astype(E4M3)
            wvp[:, 0, c2, i, :] = W[rows, 2 * C:3 * C].astype(E4M3)
            wvp[:, 1, c2, i, :] = Wp[rows, :].astype(E4M3)
    cs128 = np.zeros((128, 20 + CT * G), np.float32)
    cs128[:, 0:CT] = np.asarray(gamma, np.float32).reshape(CT, 128).T
    for ot in range(8):
        cs128[:, CT + ot] = bq[128 * ot:128 * (ot + 1)]
    for ct in range(CT):
        for c in range(128):
            cs128[c, 20 + G * ct + (128 * ct + c) // CPG] = 1.0
    cs32 = np.zeros((G, C + 1), np.float32)
    for c in range(C):
        cs32[c // CPG, c] = 1.0
    return dict(wqk=wqk, wvp=wvp), cs128, cs32, resid_bias


def kernel(x, lengths, gamma, beta, Wqkv, bqkv, Wproj, bproj):
    from concourse.bass_utils import run_bass_kernel_spmd

    x = np.asarray(x, np.float32)
    lengths = np.asarray(lengths).astype(np.int64)
    const, cs128_base, cs32, resid_bias = _prep_weights(gamma, beta, Wqkv, bqkv, Wproj, bproj)

    in_maps = []
    for s in range(B):
        ln = int(lengths[s])
        mf = (np.arange(L) < ln).astype(np.float32)
        xm = x[s] * mf[:, None]
        xmr = (xm + mf[:, None] * resid_bias[None, :]).reshape(NT, 128, C).transpose(1, 0, 2)
        xmT = np.ascontiguousarray(xm.T.reshape(CT, 128, L).transpose(1, 0, 2)).astype(BF16)
        cs128 = cs128_base.copy()
        cs128[:, 12:12 + NT] = mf.reshape(NT, 128).T
        cs32_s = cs32.copy()
        cs32_s[:, C] = 1.0 / max(ln * CPG, 1)
        m = dict(const)
        m.update(xmT=xmT, xmr=np.ascontiguousarray(xmr), cs128=cs128, cs32=cs32_s)
        in_maps.append(m)

    nc = _get_nc()
    res = run_bass_kernel_spmd(nc, in_maps, core_ids=list(range(B)))
    _CACHE["last_res"] = res
    out = np.stack([res.results[s]["out"] for s in range(B)], axis=0)
    return out.astype(np.float32)


if __name__ == "__main__":
    rng = np.random.default_rng(0)
    x = rng.standard_normal((B, L, C), dtype=np.float32)
    lengths = rng.integers(L // 2, L + 1, size=(B,))
    gamma = np.ones(C, np.float32)
    beta = np.zeros(C, np.float32)
    Wqkv = (rng.standard_normal((C, 3 * C)) * 0.02).astype(np.float32)
    bqkv = np.zeros(3 * C, np.float32)
    Wproj = (rng.standard_normal((C, C)) * 0.02).astype(np.float32)
    bproj = np.zeros(C, np.float32)
    out = kernel(x=x, lengths=lengths, gamma=gamma, beta=beta, Wqkv=Wqkv,
                 bqkv=bqkv, Wproj=Wproj, bproj=bproj)
    print("out", out.shape, out.dtype, np.abs(out).max())


# revision 20
# speedup vs baseline: 1.0244x; 1.0244x over previous
"""Trainium2 Bass kernel for nn_AttentionBlock (masked GroupNorm + jagged full attention).

Contract: kernel(**inputs) takes FULL unsharded inputs (as in reference.setup_inputs())
and returns the FULL [8, 1024, 512] fp32 output. Internally shards data-parallel over
the batch: sample b -> NeuronCore b (8 cores).

v3: fp8e4 (e4m3) DoubleRow matmuls for qkv/v/av/proj (2x contraction per instr, 0.5
cycles/row); bf16 pair-packed scores (2 concurrent PE row groups); exp on ScalarE at
1-ktile granularity with ping-pong PSUM buffers (sA/sB) so the Activation engine (the
roofline: ~1 col/cycle @1.2GHz over H*L*L elements) never stalls; cross-(pair,qc)
software pipelining (the av tail + softmax normalize of pair p run under the scores/exp
of pair p+1); batched DMAs issued from multiple engine queues to cut the launch lead-in.

Per-core dataflow (sample s):
  xmT bf16 [128,CT,L] -> GroupNorm stats (bn_stats + selector matmuls) -> xn8 e4m3
    -> qkT bf16 (DoubleRow fp8, bias on DVE)  -> v8 e4m3 (masked, + denominator row)
    -> per pair: scoresT[k,q] bf16 -> exp e4m3 (ScalarE) -> av^T DoubleRow fp8
    -> reciprocal (DVE) -> partition_broadcast (Pool) -> normalize -> attn8 e4m3
    -> proj DoubleRow fp8 + residual (x*mask + all biases, host-folded), DMA out.

Padded tokens: x host-masked to zero => xn there ~= -mean*rstd*gamma (tiny); v rows
zeroed; denominator mask-row zero. Padded q columns give ~uniform attention over valid
k (|out| ~1e-2, well under the gate); residual rows are zero.
"""

import numpy as np
import ml_dtypes
from contextlib import ExitStack

B, L, C, G, H = 8, 1024, 512, 32, 8
DH = C // H          # 64
CPG = C // G         # 16
EPS = 1e-5
NT = L // 128        # 8 token tiles
CT = C // 128        # 4 channel tiles
CT2 = C // 256       # 2 DoubleRow channel groups
QC = L // 512        # 2 query chunks

BF16 = ml_dtypes.bfloat16
E4M3 = ml_dtypes.float8_e4m3

_CACHE = {}


def _build():
    import concourse.bass as bass
    import concourse.tile as tile
    from concourse import bacc, mybir

    f32 = mybir.dt.float32
    bf16 = mybir.dt.bfloat16
    e4 = mybir.dt.float8e4
    Alu = mybir.AluOpType
    Act = mybir.ActivationFunctionType
    DR = mybir.MatmulPerfMode.DoubleRow

    nc = bacc.Bacc("TRN2", target_bir_lowering=False)

    # ---- per-core DRAM inputs (host-prepped, partition-major for single-descriptor DMA) ----
    xmT_d = nc.dram_tensor("xmT", [128, CT, L], bf16, kind="ExternalInput")
    xm_d = nc.dram_tensor("xmr", [128, NT, C], f32, kind="ExternalInput")  # x*mf + folded biases
    wqk_d = nc.dram_tensor("wqk", [128, CT2, 8, 2, 128], e4, kind="ExternalInput")
    wvp_d = nc.dram_tensor("wvp", [128, 2, CT2, 2, C], e4, kind="ExternalInput")  # [v|p] rhs tiles
    # cs128: gamma[0:4] | bqk[4:12] | vmask[12:20] | sel[20:148]
    cs128_d = nc.dram_tensor("cs128", [128, 20 + CT * G], f32, kind="ExternalInput")
    cs32_d = nc.dram_tensor("cs32", [G, C + 1], f32, kind="ExternalInput")  # selT | icnt
    out_d = nc.dram_tensor("out", [L, C], f32, kind="ExternalOutput")

    with tile.TileContext(nc) as tc, ExitStack() as ctx:
        pc = ctx.enter_context(tc.tile_pool(name="consts", bufs=1))
        pb = ctx.enter_context(tc.tile_pool(name="big", bufs=1))
        ps = ctx.enter_context(tc.tile_pool(name="psum", bufs=1, space="PSUM"))

        # ---- batched loads, spread across engine queues ----
        cs128 = pc.tile([128, 20 + CT * G], f32, tag="cs128", name="cs128")
        cs32 = pc.tile([G, C + 1], f32, tag="cs32", name="cs32")
        nc.scalar.dma_start(cs128[:], cs128_d[:, :])
        nc.scalar.dma_start(cs32[:], cs32_d[:, :])
        xmt_sb = pb.tile([128, CT, L], bf16, tag="xmT", name="xmT")
        nc.sync.dma_start(xmt_sb[:, 0:1, :], xmT_d[:, 0:1, :])
        nc.sync.dma_start(xmt_sb[:, 1:2, :], xmT_d[:, 1:2, :])
        nc.scalar.dma_start(xmt_sb[:, 2:3, :], xmT_d[:, 2:3, :])
        nc.scalar.dma_start(xmt_sb[:, 3:4, :], xmT_d[:, 3:4, :])
        # gate the bulk loads behind xmT so they don't contend with the DMA
        # engines while GroupNorm's input streams in (GN is the critical path)
        gate = pb.tile([1, 8], bf16, tag="gate", name="gate")
        nc.gpsimd.tensor_copy(gate[:], xmt_sb[0:1, 3, 0:8])
        wqk_sb = pc.tile([128, CT2, 8, 2, 128], e4, tag="wqk", name="wqk")
        nc.gpsimd.dma_start(wqk_sb[:], wqk_d[:, :, :, :, :])
        wvp_sb = pc.tile([128, 2, CT2, 2, C], e4, tag="wvp", name="wvp")
        nc.gpsimd.dma_start(wvp_sb[:], wvp_d[:, :, :, :, :])
        xm_sb = pb.tile([128, NT, C], f32, tag="xm", name="xm")
        nc.gpsimd.dma_start(xm_sb[:, 0:4, :], xm_d[:, 0:4, :])
        nc.gpsimd.dma_start(xm_sb[:, 4:8, :], xm_d[:, 4:8, :])

        gam = cs128[:, 0:CT]
        bqk = cs128[:, CT:CT + 8]
        vmask = cs128[:, 12:12 + NT]
        sel = [cs128[:, 20 + G * ct:20 + G * (ct + 1)] for ct in range(CT)]
        selT = cs32[:, 0:C]
        icnt = cs32[:, C:C + 1]

        # ---- Phase 1: GroupNorm (stats over valid tokens; zeros from host masking) ----
        smm = [pb.tile([128, 2], f32, tag=f"smm{t}", name=f"smm{t}") for t in range(CT)]
        ps_g = ps.tile([G, 2], f32, tag="pA", name="psg")
        for t in range(CT):
            bns = pb.tile([128, 2, 6], f32, tag="bns", name="bns")
            nc.vector.bn_stats(bns[:, 0, :], xmt_sb[:, t, 0:512])
            nc.vector.bn_stats(bns[:, 1, :], xmt_sb[:, t, 512:1024])
            mv = pb.tile([128, 2], f32, tag="mv", name="mv")
            nc.vector.bn_aggr(mv[:], bns[:])
            sq = pb.tile([128, 1], f32, tag="sq", name="sq")
            nc.vector.tensor_mul(sq[:], mv[:, 0:1], mv[:, 0:1])
            # smm = [sum(x), sum(x^2)] recovered from mean/var over all 1024 (incl. zeros)
            nc.vector.tensor_scalar(smm[t][:, 0:1], mv[:, 0:1], float(L), None, Alu.mult)
            nc.vector.tensor_scalar(smm[t][:, 1:2], mv[:, 1:2], sq[:, 0:1], float(L), Alu.add, Alu.mult)
        for t in range(CT):
            nc.tensor.matmul(ps_g[:], sel[t], smm[t][:], start=(t == 0), stop=(t == CT - 1))
        grp = pb.tile([G, 2], f32, tag="grp", name="grp")      # [mean_g, rstd_g]
        ex2 = pb.tile([G, 1], f32, tag="ex2", name="ex2")
        nc.vector.tensor_scalar(grp[:, 0:1], ps_g[:, 0:1], icnt, None, Alu.mult)
        nc.vector.tensor_scalar(ex2[:], ps_g[:, 1:2], icnt, None, Alu.mult)
        mm2 = pb.tile([G, 1], f32, tag="mm2", name="mm2")
        nc.vector.tensor_mul(mm2[:], grp[:, 0:1], grp[:, 0:1])
        var = pb.tile([G, 1], f32, tag="var", name="var")
        nc.vector.tensor_tensor(var[:], ex2[:], mm2[:], Alu.subtract)
        sd = pb.tile([G, 1], f32, tag="sd", name="sd")
        eps_sb = pb.tile([G, 1], f32, tag="eps", name="eps")
        nc.vector.memset(eps_sb[:], EPS)
        nc.scalar.activation(sd[:], var[:], Act.Sqrt, bias=eps_sb[:], scale=1.0)
        # preload the Exp activation table while GroupNorm finishes on DVE
        # (reads sd so the scheduler keeps it AFTER the Sqrt's table load)
        dummy = pb.tile([G, 1], f32, tag="dummy", name="dummy")
        nc.scalar.activation(dummy[:], sd[:], Act.Exp, bias=0.0, scale=1.0)
        nc.vector.reciprocal(grp[:, 1:2], sd[:])

        # xn8[c2][p, i, tok] = xn(channel 256*c2 + 128*i + p, tok) in e4m3
        xn8_sb = [pb.tile([128, 2, L], e4, tag=f"xn8_{c2}", name=f"xn8_{c2}") for c2 in range(CT2)]
        rg_sb = pb.tile([128, CT], f32, tag="rg", name="rg")
        chst = [pb.tile([128, 2], f32, tag=f"chst{t}", name=f"chst{t}") for t in range(CT)]
        for t in range(CT):
            ps_b = ps.tile([128, 2], f32, tag="pB", name="psb")
            nc.tensor.matmul(ps_b[:], selT[:, 128 * t:128 * (t + 1)], grp[:], start=True, stop=True)
            nc.vector.tensor_copy(chst[t][:], ps_b[:])
            nc.vector.tensor_mul(rg_sb[:, t:t + 1], chst[t][:, 1:2], gam[:, t:t + 1])
            nc.vector.tensor_scalar(xn8_sb[t // 2][:, t % 2, :], xmt_sb[:, t, :],
                                    chst[t][:, 0:1], rg_sb[:, t:t + 1],
                                    Alu.subtract, Alu.mult)

        qkT_sb = [pb.tile([128, L], bf16, tag=f"qkT{ot}", name=f"qkT{ot}") for ot in range(8)]
        # v8[p, g, u, h, d]: v of token 128*(2g+u)+p, head h, dh d; d==64 is the mask
        # row; stride 72 keeps the DoubleRow ldweights step 16B-aligned
        v8_sb = pb.tile([128, NT // 2, 2, H, 72], e4, tag="v8", name="v8")
        attn8_sb = [pb.tile([128, 2, L], e4, tag=f"attn8_{c2}", name=f"attn8_{c2}") for c2 in range(CT2)]
        bcast_sb = [pb.tile([64, 512], f32, tag=f"bcast{j}", name=f"bcast{j}", bufs=2) for j in range(2)]

        def emit_qk1(ot, qc):
            qs = slice(512 * qc, 512 * (qc + 1))
            pq = ps.tile([128, 512], f32, tag=("pA" if qc == 0 else "pB"), name="pq")
            for c2 in range(CT2):
                nc.tensor.matmul(pq[:], wqk_sb[:, c2, ot, :, :], xn8_sb[c2][:, :, qs],
                                 start=(c2 == 0), stop=(c2 == CT2 - 1), perf_mode=DR)
            nc.vector.tensor_scalar(qkT_sb[ot][:, qs], pq[:],
                                    bqk[:, ot:ot + 1], None, Alu.add)

        def emit_v1(kt):
            pv = ps.tile([128, 512], f32, tag=("pA" if kt % 2 == 0 else "pB"), name="pv")
            for c2 in range(CT2):
                nc.tensor.matmul(pv[:], xn8_sb[c2][:, :, 128 * kt:128 * (kt + 1)],
                                 wvp_sb[:, 0, c2, :, :],
                                 start=(c2 == 0), stop=(c2 == CT2 - 1), perf_mode=DR)
            g, u = kt // 2, kt % 2
            nc.vector.tensor_scalar(v8_sb[:, g, u, :, 0:DH],
                                    pv[:].rearrange("p (h d) -> p h d", h=H),
                                    vmask[:, kt:kt + 1], None, Alu.mult)
            nc.vector.tensor_copy(v8_sb[:, g, u, :, DH],
                                  vmask[:, kt:kt + 1].to_broadcast((128, H)))

        def emit_proj(qt, ptag, otag):
            po = ps.tile([128, 512], f32, tag=ptag, name="po")
            for c2 in range(CT2):
                nc.tensor.matmul(po[:], attn8_sb[c2][:, :, 128 * qt:128 * (qt + 1)],
                                 wvp_sb[:, 1, c2, :, :],
                                 start=(c2 == 0), stop=(c2 == CT2 - 1), perf_mode=DR)
            o_sb = pb.tile([128, C], f32, tag=otag, name=otag)
            nc.vector.tensor_add(o_sb[:], po[:], xm_sb[:, qt, :])
            nc.sync.dma_start(out_d[128 * qt:128 * (qt + 1), :], o_sb[:])

        def emit_attn(p, qc, fin_prev, pre=(), fillers={}):
            kT = qkT_sb[4 + p]
            qT = qkT_sb[p]
            qs = slice(512 * qc, 512 * (qc + 1))
            avs = [ps.tile([DH + 1, 512], f32, tag=("avA" if j == 0 else "avB"), name=f"av{j}")
                   for j in range(2)]
            # expT2[p, j, kt, q]: exp'd transposed scores for the two heads of this pair
            expT2 = pb.tile([128, 2, NT, 512], e4, tag="expT2", name="expT2", bufs=2)

            def sc_exp(kt):
                ks = slice(128 * kt, 128 * (kt + 1))
                s = ps.tile([128, 2, 512], f32, tag=("sA" if kt % 2 == 0 else "sB"), name="s")
                nc.tensor.matmul(s[:, 0, :], kT[0:64, ks], qT[0:64, qs], start=True, stop=True)
                nc.tensor.matmul(s[:, 1, :], kT[64:128, ks], qT[64:128, qs], start=True, stop=True)
                nc.scalar.activation(expT2[:, :, kt, :], s[:], Act.Exp, bias=0.0, scale=0.125)

            def av_group(g):
                for j in range(2):
                    h = 2 * p + j
                    nc.tensor.matmul(avs[j][:], v8_sb[:, g, :, h, 0:DH + 1],
                                     expT2[:, j, 2 * g:2 * g + 2, :],
                                     start=(g == 0), stop=(g == NT // 2 - 1), perf_mode=DR)

            # cross-stage software pipeline: finish work of the previous (pair, qc)
            # runs under this pair's first scores/exp; filler units (qk/v/proj) are
            # threaded one-per-ktile to keep the PE fed without delaying scores
            sc_exp(0)
            if len(pre) > 0:
                pre[0]()
            sc_exp(1)
            if len(pre) > 1:
                pre[1]()
            if fin_prev is not None:
                fin_prev()
            for kt in range(2, NT):
                sc_exp(kt)
                if kt in fillers:
                    fillers[kt]()
                if kt % 2 == 1:
                    av_group((kt - 3) // 2)   # g = 0,1,2 at kt = 3,5,7

            def finish():
                av_group(NT // 2 - 1)
                for j in range(2):
                    # custom DVE/Pool ops can't read PSUM: copy the denominator row out
                    den = pb.tile([1, 512], f32, tag=f"den{j}", name=f"den{j}", bufs=2)
                    nc.vector.tensor_copy(den[:], avs[j][DH:DH + 1, :])
                    rec = pb.tile([1, 512], f32, tag=f"rec{j}", name=f"rec{j}", bufs=2)
                    nc.vector.reciprocal_approx_fast(rec[:], den[:])
                    nc.gpsimd.partition_broadcast(bcast_sb[j][:], rec[:])
                    nc.vector.tensor_tensor(attn8_sb[p // 2][64 * j:64 * (j + 1), p % 2, qs],
                                            avs[j][0:DH, :], bcast_sb[j][:], Alu.mult)
            return finish

        def qkf(ot, qc):
            return lambda: emit_qk1(ot, qc)

        def vf(kt):
            return lambda: emit_v1(kt)

        def pjf(qt):
            return lambda: emit_proj(qt, "pA" if qt % 2 == 0 else "pB", f"o{qt % 2}")

        # ---- emission schedule (qc0 tiles first: ktile 0-3 scores need only qc0) ----
        for ot, qc in [(0, 0), (4, 0), (4, 1), (0, 1)]:
            emit_qk1(ot, qc)
        fin = emit_attn(0, 0, None, fillers={k + 2: vf(k) for k in range(6)})
        fin = emit_attn(0, 1, fin, pre=[vf(6), vf(7)],
                        fillers={2: qkf(1, 0), 3: qkf(1, 1), 4: qkf(5, 0), 5: qkf(5, 1)})
        fin = emit_attn(1, 0, fin,
                        fillers={2: qkf(2, 0), 3: qkf(2, 1), 4: qkf(6, 0), 5: qkf(6, 1)})
        fin = emit_attn(1, 1, fin,
                        fillers={2: qkf(3, 0), 3: qkf(3, 1), 4: qkf(7, 0), 5: qkf(7, 1)})
        fin = emit_attn(2, 0, fin)
        fin = emit_attn(2, 1, fin)
        fin = emit_attn(3, 0, fin)
        fin = emit_attn(3, 1, fin,
                        fillers={4: pjf(0), 5: pjf(1), 6: pjf(2), 7: pjf(3)})
        fin()
        for qt, ptag, otag in [(4, "pA", "o0"), (5, "pB", "o1"),
                               (6, "sA", "o2"), (7, "sB", "o3")]:
            emit_proj(qt, ptag, otag)

    nc.compile()
    return nc


def _get_nc():
    if "nc" not in _CACHE:
        _CACHE["nc"] = _build()
    return _CACHE["nc"]


def _prep_weights(gamma, beta, Wqkv, bqkv, Wproj, bproj):
    """Host-side constant prep shared across cores."""
    W = np.asarray(Wqkv, np.float32)
    bq = np.asarray(bqkv, np.float32) + np.asarray(beta, np.float32) @ W   # fold beta
    Wp = np.asarray(Wproj, np.float32)
    bv = bq[2 * C:3 * C]
    # residual-side constant: bproj + bv @ Wproj (added to masked rows on host)
    resid_bias = np.asarray(bproj, np.float32) + bv @ Wp

    # DoubleRow lhsT/rhs tiles: contraction channel (c2, i, p) = 256*c2 + 128*i + p
    wqk = np.zeros((128, CT2, 8, 2, 128), E4M3)
    wvp = np.zeros((128, 2, CT2, 2, C), E4M3)
    for c2 in range(CT2):
        for i in range(2):
            rows = slice(256 * c2 + 128 * i, 256 * c2 + 128 * (i + 1))
            for ot in range(8):
                wqk[:, c2, ot, i, :] = W[rows, 128 * ot:128 * (ot + 1)].astype(E4M3)
            wvp[:, 0, c2, i, :] = W[rows, 2 * C:3 * C].astype(E4M3)
            wvp[:, 1, c2, i, :] = Wp[rows, :].astype(E4M3)
    cs128 = np.zeros((128, 20 + CT * G), np.float32)
    cs128[:, 0:CT] = np.asarray(gamma, np.float32).reshape(CT, 128).T
    for ot in range(8):
        cs128[:, CT + ot] = bq[128 * ot:128 * (ot + 1)]
    for ct in range(CT):
        for c in range(128):
            cs128[c, 20 + G * ct + (128 * ct + c) // CPG] = 1.0
    cs32 = np.zeros((G, C + 1), np.float32)
    for c in range(C):
        cs32[c // CPG, c] = 1.0
    return dict(wqk=wqk, wvp=wvp), cs128, cs32, resid_bias


def kernel(x, lengths, gamma, beta, Wqkv, bqkv, Wproj, bproj):
    from concourse.bass_utils import run_bass_kernel_spmd

    x = np.asarray(x, np.float32)
    lengths = np.asarray(lengths).astype(np.int64)
    const, cs128_base, cs32, resid_bias = _prep_weights(gamma, beta, Wqkv, bqkv, Wproj, bproj)

    in_maps = []
    for s in range(B):
        ln = int(lengths[s])
        mf = (np.arange(L) < ln).astype(np.float32)
        xm = x[s] * mf[:, None]
        xmr = (xm + mf[:, None] * resid_bias[None, :]).reshape(NT, 128, C).transpose(1, 0, 2)
        xmT = np.ascontiguousarray(xm.T.reshape(CT, 128, L).transpose(1, 0, 2)).astype(BF16)
        cs128 = cs128_base.copy()
        cs128[:, 12:12 + NT] = mf.reshape(NT, 128).T
        cs32_s = cs32.copy()
        cs32_s[:, C] = 1.0 / max(ln * CPG, 1)
        m = dict(const)
        m.update(xmT=xmT, xmr=np.ascontiguousarray(xmr), cs128=cs128, cs32=cs32_s)
        in_maps.append(m)

    nc = _get_nc()
    res = run_bass_kernel_spmd(nc, in_maps, core_ids=list(range(B)))
    _CACHE["last_res"] = res
    out = np.stack([res.results[s]["out"] for s in range(B)], axis=0)
    return out.astype(np.float32)


if __name__ == "__main__":
    rng = np.random.default_rng(0)
    x = rng.standard_normal((B, L, C), dtype=np.float32)
    lengths = rng.integers(L // 2, L + 1, size=(B,))
    gamma = np.ones(C, np.float32)
    beta = np.zeros(C, np.float32)
    Wqkv = (rng.standard_normal((C, 3 * C)) * 0.02).astype(np.float32)
    bqkv = np.zeros(3 * C, np.float32)
    Wproj = (rng.standard_normal((C, C)) * 0.02).astype(np.float32)
    bproj = np.zeros(C, np.float32)
    out = kernel(x=x, lengths=lengths, gamma=gamma, beta=beta, Wqkv=Wqkv,
                 bqkv=bqkv, Wproj=Wproj, bproj=bproj)
    print("out", out.shape, out.dtype, np.abs(out).max())
